# revision 19
# baseline (speedup 1.0000x reference)
"""GQA kernel for Trainium2, 8 NeuronCores.

Sharding: core c = b*4 + kv  (b in {0,1} data-parallel over batch,
kv in {0..3} tensor-parallel over the 4 KV head groups; each core owns
4 Q heads + 1 KV head). Each core computes a partial output
x[b] @ Wq[:,kv] -> attention -> @ Wo[kv rows]; host sums the 4 partials
per batch (the row-sharded-Wo all-reduce).

v3 (all matmul operands bf16; PSUM accumulation f32; one PSUM
accumulation group per bank - groups may NOT share banks):

Phase 1, per 256-row eighth: K projection, V projection in natural
layout (bf16 N=128 matmuls - no PE transposes), 4 Q projections; each
followed by in-layout RMSNorm+RoPE where rotate-half is a
signed-permutation matmul on PE (no SBUF-SBUF DMAs). xt streamed in
bf16 with depth-2 prefetch; weights staged wk->xt0->wv->wq->tables->wo.

Phase 2, per 512-col q block J: S^T[k,q] = K Q^T per 128-key block,
exp on Act, softmax denominator via ones-matmul accumulated in PSUM,
O^T accumulated over key blocks; diagonal key blocks column-shrunk
(bf16 matmuls have no minimum-N penalty). Output projection of block
J-1 is interleaved between heads of block J, streamed out in
[128,512] chunks.
"""

import numpy as np

B, T, D = 2, 2048, 2048
NH, NKV, HD = 16, 4, 128
GQ = NH // NKV            # 4 q heads per kv head
HQ = GQ * HD              # 512 q-dim per core
ROPE_BASE = 500000.0
EPS = 1e-5
SCALE = 1.0 / np.sqrt(HD)
NE = 8                    # T eighths (phase 1)
ET = T // NE              # 256
NDC = D // 128            # 16 contraction chunks
NJ = 4                    # phase-2 q blocks
JW = T // NJ              # 512

_cached = {}


def _build_program():
    import concourse.bacc as bacc
    import concourse.mybir as mybir
    from concourse import tile
    from concourse.bass import ts

    f32 = mybir.dt.float32
    bf16 = mybir.dt.bfloat16
    AF = mybir.ActivationFunctionType

    nc = bacc.Bacc()

    xt = nc.dram_tensor("xt", [D, T], bf16, kind="ExternalInput")
    wq = nc.dram_tensor("wq", [128, NDC, HQ], bf16, kind="ExternalInput")
    wk = nc.dram_tensor("wk", [128, NDC, HD], bf16, kind="ExternalInput")
    wv = nc.dram_tensor("wv", [128, NDC, HD], bf16, kind="ExternalInput")
    wo = nc.dram_tensor("wo", [128, GQ, D], bf16, kind="ExternalInput")
    cosd = nc.dram_tensor("cos", [HD, T], bf16, kind="ExternalInput")
    sind = nc.dram_tensor("sin", [HD, T], bf16, kind="ExternalInput")
    permd = nc.dram_tensor("perm", [128, 128], bf16, kind="ExternalInput")
    trid = nc.dram_tensor("tri", [128, 128], bf16, kind="ExternalInput")
    onesd = nc.dram_tensor("ones", [128, 128], bf16, kind="ExternalInput")
    onesnd = nc.dram_tensor("onesn", [128, 128], bf16, kind="ExternalInput")
    wqcd = nc.dram_tensor("wqc", [HD, 1], f32, kind="ExternalInput")
    wkcd = nc.dram_tensor("wkc", [HD, 1], f32, kind="ExternalInput")
    wqed = nc.dram_tensor("wqe", [HD, 1], f32, kind="ExternalInput")
    wked = nc.dram_tensor("wke", [HD, 1], f32, kind="ExternalInput")
    outd = nc.dram_tensor("out", [T, D], f32, kind="ExternalOutput")

    xtr = xt.rearrange("(c p) t -> p c t", p=128)

    with tile.TileContext(nc) as tc:
        with (
            tc.tile_pool(name="A", bufs=1) as A,
            tc.tile_pool(name="BX", bufs=3) as BX,
            tc.tile_pool(name="TMP", bufs=3) as TMP,
            tc.tile_pool(name="CP", bufs=6) as CP,
            tc.tile_pool(name="CT", bufs=4) as CT,
            tc.tile_pool(name="CO", bufs=8) as CO,
        ):
            # persistent tiles
            QT = A.tile([128, GQ, T], bf16, tag="QT")
            KT = A.tile([128, T], bf16, tag="KT")
            Vn = A.tile([128, NE * 2, HD], bf16, tag="Vn")
            wq_sb = A.tile([128, NDC, HQ], bf16, tag="wq")
            wk_sb = A.tile([128, NDC, HD], bf16, tag="wk")
            wv_sb = A.tile([128, NDC, HD], bf16, tag="wv")
            wo_sb = A.tile([128, GQ, D], bf16, tag="wo")
            cos_sb = A.tile([128, T], bf16, tag="cos")
            sin_sb = A.tile([128, T], bf16, tag="sin")
            perm_sb = A.tile([128, 128], bf16, tag="perm")
            tri_sb = A.tile([128, 128], bf16, tag="tri")
            ones_sb = A.tile([128, 128], bf16, tag="ones")
            onesn_sb = A.tile([128, 128], bf16, tag="onesn")
            wqc = A.tile([128, 1], f32, tag="wqc")
            wkc = A.tile([128, 1], f32, tag="wkc")
            wqe = A.tile([128, 1], f32, tag="wqe")
            wke = A.tile([128, 1], f32, tag="wke")
            OTJ = [A.tile([128, GQ, JW], bf16, tag="OTa", name="OTa"),
                   A.tile([128, GQ, JW], bf16, tag="OTb", name="OTb")]

            # Dummy first activation: a Sqrt, so the act-table pass loads the
            # sqrt set (which also holds square+copy) once at startup instead
            # of loading the square set first and swapping mid-phase-1.
            warm = A.tile([128, 1], f32, tag="warm")
            nc.vector.memset(warm, 1.0)
            nc.scalar.activation(warm, warm, AF.Sqrt)

            # staged preload: wk -> xt0 -> wv -> wq -> tables -> xt1 -> wo
            nc.sync.dma_start(out=wk_sb, in_=wk[:, :, :])
            xt_bufs = {}
            xt_bufs[0] = BX.tile([128, NDC, ET], bf16, tag="xt", name="xt0")
            for g in range(4):
                nc.sync.dma_start(out=xt_bufs[0][:, ts(g, 4), :],
                                  in_=xtr[:, ts(g, 4), 0:ET])
            nc.sync.dma_start(out=wv_sb, in_=wv[:, :, :])
            nc.sync.dma_start(out=wq_sb[:, 0:8, :], in_=wq[:, 0:8, :])
            nc.sync.dma_start(out=wq_sb[:, 8:16, :], in_=wq[:, 8:16, :])
            nc.sync.dma_start(out=cos_sb, in_=cosd[:, :])
            nc.sync.dma_start(out=sin_sb, in_=sind[:, :])
            nc.sync.dma_start(out=perm_sb, in_=permd[:, :])
            nc.sync.dma_start(out=tri_sb, in_=trid[:, :])
            nc.sync.dma_start(out=ones_sb, in_=onesd[:, :])
            nc.sync.dma_start(out=onesn_sb, in_=onesnd[:, :])
            nc.sync.dma_start(out=wqc, in_=wqcd[:, :])
            nc.sync.dma_start(out=wkc, in_=wkcd[:, :])
            nc.sync.dma_start(out=wqe, in_=wqed[:, :])
            nc.sync.dma_start(out=wke, in_=wked[:, :])
            xt_bufs[1] = BX.tile([128, NDC, ET], bf16, tag="xt", name="xt1")
            for g in range(2):
                nc.sync.dma_start(out=xt_bufs[1][:, ts(g, 8), :],
                                  in_=xtr[:, ts(g, 8), ET:2 * ET])
            nc.sync.dma_start(out=wo_sb[:, 0:2, :], in_=wo[:, 0:2, :])
            nc.sync.dma_start(out=wo_sb[:, 2:4, :], in_=wo[:, 2:4, :])

            # ---------------- phase 1: projections ----------------
            with (
                tc.tile_pool(name="PP", bufs=4, space="PSUM") as PP,
                tc.tile_pool(name="PL", bufs=2, space="PSUM") as PL,
                tc.tile_pool(name="PR", bufs=2, space="PSUM") as PR,
            ):
                def normrope(cpsum, wcol, wbias, sl, out_sl):
                    """RMSNorm (partition-dim mean via ones-matmul) + norm
                    weight + RoPE (rotate-half via signed-permutation matmul
                    on PE). Writes bf16 out_sl [128, ET]."""
                    sq = TMP.tile([128, ET], bf16, tag="sq")
                    nc.scalar.activation(sq, cpsum, AF.Square)
                    l2 = PL.tile([128, ET], f32, tag="l2")
                    nc.tensor.matmul(l2, onesn_sb, sq, start=True, stop=True)
                    sv = TMP.tile([128, ET], f32, tag="sv")
                    nc.scalar.activation(sv, l2, AF.Sqrt, scale=wcol, bias=wbias)
                    rc = TMP.tile([128, ET], f32, tag="rc")
                    nc.vector.reciprocal(rc, sv)
                    qn = TMP.tile([128, ET], bf16, tag="qn")
                    nc.vector.tensor_mul(qn, cpsum, rc)
                    qr = PR.tile([128, ET], f32, tag="qr")
                    nc.tensor.matmul(qr, perm_sb, qn, start=True, stop=True)
                    t1 = TMP.tile([128, ET], bf16, tag="t1")
                    nc.vector.tensor_mul(t1, qn, cos_sb[:, sl])
                    t2 = TMP.tile([128, ET], bf16, tag="t2")
                    nc.vector.tensor_mul(t2, qr, sin_sb[:, sl])
                    nc.vector.tensor_add(out_sl, t1, t2)

                for e in range(NE):
                    sl = ts(e, ET)
                    if e + 2 < NE:
                        xt_bufs[e + 2] = BX.tile([128, NDC, ET], bf16,
                                                 tag="xt", name=f"xt{e + 2}")
                        for g in range(2):
                            nc.sync.dma_start(
                                out=xt_bufs[e + 2][:, ts(g, 8), :],
                                in_=xtr[:, ts(g, 8), (e + 2) * ET:(e + 3) * ET])
                    xt_t = xt_bufs.pop(e)
                    # K projection
                    kp = PP.tile([128, ET], f32, tag="pp", name="kp")
                    for c in range(NDC):
                        nc.tensor.matmul(kp, wk_sb[:, c, :], xt_t[:, c, :],
                                         start=(c == 0), stop=(c == NDC - 1))
                    # V natural-layout projections
                    vps = []
                    for i in range(2):
                        vp = PP.tile([128, HD], f32, tag="pp", name=f"vp{i}")
                        for c in range(NDC):
                            nc.tensor.matmul(vp, xt_t[:, c, ts(i, 128)],
                                             wv_sb[:, c, :],
                                             start=(c == 0), stop=(c == NDC - 1))
                        vps.append(vp)
                    normrope(kp, wkc, wke, sl, KT[:, sl])
                    for i in range(2):
                        nc.scalar.activation(Vn[:, 2 * e + i, :], vps[i],
                                             AF.Copy)
                    # Q heads
                    for h in range(GQ):
                        qp = PP.tile([128, ET], f32, tag="pp", name=f"qp{h}")
                        for c in range(NDC):
                            nc.tensor.matmul(qp, wq_sb[:, c, ts(h, 128)],
                                             xt_t[:, c, :],
                                             start=(c == 0), stop=(c == NDC - 1))
                        normrope(qp, wqc, wqe, sl, QT[:, h, sl])

            # Dummy Exp so the exp act-table load overlaps the phase-1 drain
            # instead of stalling the first attention block.
            nc.scalar.activation(warm, warm, AF.Exp)

            # ---------------- phase 2: attention + out projection ---------
            with (
                tc.tile_pool(name="PS2", bufs=3, space="PSUM") as PS2,
                tc.tile_pool(name="PLP", bufs=1, space="PSUM") as PLP,
                tc.tile_pool(name="POP", bufs=1, space="PSUM") as POP,
                tc.tile_pool(name="PS3", bufs=3, space="PSUM") as PS3,
            ):
                def outproj_chunk(Jm, c, spread=False):
                    """Output projection: column chunk c (of 4) for the four
                    128-row q tiles of block Jm; streams each [128,512] chunk
                    straight out. spread=True rotates oup allocations through
                    the idle lp/op banks too (used for the final block, which
                    has no attention work to hide the bank latency)."""
                    for qt in range(4):
                        qtg = 4 * Jm + qt
                        if spread and qt == 1:
                            oup = PLP.tile([128, 512], f32, tag="lp")
                        elif spread and qt == 2:
                            oup = POP.tile([128, 512], f32, tag="op")
                        else:
                            oup = PS3.tile([128, 512], f32, tag="oup")
                        for hc in range(GQ):
                            nc.tensor.matmul(oup, OTJ[Jm % 2][:, hc, ts(qt, 128)],
                                             wo_sb[:, hc, ts(c, 512)],
                                             start=(hc == 0), stop=(hc == GQ - 1))
                        oc = CO.tile([128, 512], f32, tag="oc")
                        nc.scalar.activation(oc, oup, AF.Copy)
                        nc.sync.dma_start(
                            out=outd[qtg * 128:(qtg + 1) * 128,
                                     c * 512:(c + 1) * 512],
                            in_=oc)

                for J in range(NJ):
                    nkb = 4 * J + 4
                    for h in range(GQ):
                        lp = PLP.tile([128, JW], f32, tag="lp")
                        op = POP.tile([128, JW], f32, tag="op")
                        for kb in range(nkb):
                            r = kb - 4 * J  # >= 0 on the diagonal blocks
                            c0 = 128 * r if r > 0 else 0
                            sp = PS2.tile([128, JW], f32, tag="s")
                            nc.tensor.matmul(sp[:, c0:JW], KT[:, ts(kb, 128)],
                                             QT[:, h, J * JW + c0:(J + 1) * JW],
                                             start=True, stop=True,
                                             skip_group_check=True)
                            P = CP.tile([128, JW], bf16, tag="p")
                            nc.scalar.activation(P[:, c0:JW], sp[:, c0:JW],
                                                 AF.Exp, scale=SCALE)
                            if r >= 0:
                                nc.vector.tensor_mul(
                                    P[:, 128 * r:128 * r + 128],
                                    P[:, 128 * r:128 * r + 128], tri_sb)
                            nc.tensor.matmul(lp[:, c0:JW], ones_sb, P[:, c0:JW],
                                             start=(kb == 0), stop=(kb == nkb - 1),
                                             skip_group_check=True)
                            nc.tensor.matmul(op[:, c0:JW], Vn[:, kb, :],
                                             P[:, c0:JW],
                                             start=(kb == 0), stop=(kb == nkb - 1),
                                             skip_group_check=True)
                        rc2 = CT.tile([128, JW], f32, tag="rc2")
                        nc.vector.reciprocal(rc2, lp)
                        nc.vector.tensor_mul(OTJ[J % 2][:, h, :], op, rc2)
                        if J > 0:
                            outproj_chunk(J - 1, h)
                for c in range(4):
                    outproj_chunk(NJ - 1, c)

    nc.finalize()
    return nc


def _host_consts():
    import ml_dtypes
    bf16 = ml_dtypes.bfloat16
    inv = 1.0 / (ROPE_BASE ** (np.arange(0, HD, 2, dtype=np.float64) / HD))
    freqs = np.outer(np.arange(T, dtype=np.float64), inv)
    emb = np.concatenate([freqs, freqs], axis=-1)          # [T, HD]
    cosT = np.ascontiguousarray(np.cos(emb).T.astype(bf16))  # [HD, T]
    sinT = np.ascontiguousarray(np.sin(emb).T.astype(bf16))
    # signed permutation for rotate-half: qr[m] = -qn[m+64] (m<64), qn[m-64]
    perm = np.zeros((128, 128), np.float32)
    for m in range(64):
        perm[m + 64, m] = -1.0
        perm[m, m + 64] = 1.0
    perm = perm.astype(bf16)
    k = np.arange(128)[:, None]
    q = np.arange(128)[None, :]
    tri = (k <= q).astype(bf16)
    ones = np.ones((128, 128), bf16)
    onesn = np.full((128, 128), 1.0 / HD, bf16)
    return cosT, sinT, perm, tri, ones, onesn


def kernel(x, Wq, Wk, Wv, Wo, q_norm_w, k_norm_w):
    import ml_dtypes
    from concourse.bass_utils import run_bass_kernel_spmd
    bf16 = ml_dtypes.bfloat16

    if "nc" not in _cached:
        _cached["nc"] = _build_program()
        _cached["consts"] = _host_consts()
    nc = _cached["nc"]
    cosT, sinT, perm, tri, ones, onesn = _cached["consts"]

    x = np.asarray(x, np.float32)
    Wq = np.asarray(Wq, np.float32)
    Wk = np.asarray(Wk, np.float32)
    Wv = np.asarray(Wv, np.float32)
    Wo = np.asarray(Wo, np.float32)
    qwf = np.asarray(q_norm_w, np.float64).reshape(HD, 1)
    kwf = np.asarray(k_norm_w, np.float64).reshape(HD, 1)
    qw = np.ascontiguousarray((1.0 / qwf ** 2).astype(np.float32))
    kw = np.ascontiguousarray((1.0 / kwf ** 2).astype(np.float32))
    qwe = np.ascontiguousarray((EPS / qwf ** 2).astype(np.float32))
    kwe = np.ascontiguousarray((EPS / kwf ** 2).astype(np.float32))

    xTb = [np.ascontiguousarray(x[b].T.astype(bf16)) for b in range(B)]
    in_maps = []
    for core in range(8):
        b, kv = divmod(core, NKV)
        wq_p = np.ascontiguousarray(
            Wq[:, kv * HQ:(kv + 1) * HQ].reshape(NDC, 128, HQ)
            .transpose(1, 0, 2).astype(bf16))
        wk_p = np.ascontiguousarray(
            Wk[:, kv * HD:(kv + 1) * HD].reshape(NDC, 128, HD)
            .transpose(1, 0, 2).astype(bf16))
        wv_p = np.ascontiguousarray(
            Wv[:, kv * HD:(kv + 1) * HD].reshape(NDC, 128, HD)
            .transpose(1, 0, 2).astype(bf16))
        wo_p = np.ascontiguousarray(
            Wo[kv * HQ:(kv + 1) * HQ, :].reshape(GQ, 128, D)
            .transpose(1, 0, 2).astype(bf16))
        in_maps.append({
            "xt": xTb[b],
            "wq": wq_p, "wk": wk_p, "wv": wv_p, "wo": wo_p,
            "cos": cosT, "sin": sinT,
            "perm": perm, "tri": tri, "ones": ones, "onesn": onesn,
            "wqc": qw, "wkc": kw, "wqe": qwe, "wke": kwe,
        })
    res = run_bass_kernel_spmd(nc, in_maps, list(range(8)))
    out = np.zeros((B, T, D), np.float64)
    for core in range(8):
        b = core // NKV
        out[b] += res.results[core]["out"].astype(np.float64)
    return out.astype(np.float32)


# revision 20
# speedup vs baseline: 1.0057x; 1.0057x over previous
"""GQA kernel for Trainium2, 8 NeuronCores.

Sharding: core c = b*4 + kv  (b in {0,1} data-parallel over batch,
kv in {0..3} tensor-parallel over the 4 KV head groups; each core owns
4 Q heads + 1 KV head). Each core computes a partial output
x[b] @ Wq[:,kv] -> attention -> @ Wo[kv rows]; host sums the 4 partials
per batch (the row-sharded-Wo all-reduce).

v3 (all matmul operands bf16; PSUM accumulation f32; one PSUM
accumulation group per bank - groups may NOT share banks):

Phase 1, per 256-row eighth: K projection, V projection in natural
layout (bf16 N=128 matmuls - no PE transposes), 4 Q projections; each
followed by in-layout RMSNorm+RoPE where rotate-half is a
signed-permutation matmul on PE (no SBUF-SBUF DMAs). xt streamed in
bf16 with depth-2 prefetch; weights staged wk->xt0->wv->wq->tables->wo.

Phase 2, per 512-col q block J: S^T[k,q] = K Q^T per 128-key block,
exp on Act, softmax denominator via ones-matmul accumulated in PSUM,
O^T accumulated over key blocks; diagonal key blocks column-shrunk
(bf16 matmuls have no minimum-N penalty). Output projection of block
J-1 is interleaved between heads of block J, streamed out in
[128,512] chunks.
"""

import numpy as np

B, T, D = 2, 2048, 2048
NH, NKV, HD = 16, 4, 128
GQ = NH // NKV            # 4 q heads per kv head
HQ = GQ * HD              # 512 q-dim per core
ROPE_BASE = 500000.0
EPS = 1e-5
SCALE = 1.0 / np.sqrt(HD)
NE = 8                    # T eighths (phase 1)
ET = T // NE              # 256
NDC = D // 128            # 16 contraction chunks
NJ = 4                    # phase-2 q blocks
JW = T // NJ              # 512

_cached = {}


def _build_program():
    import concourse.bacc as bacc
    import concourse.mybir as mybir
    from concourse import tile
    from concourse.bass import ts

    f32 = mybir.dt.float32
    bf16 = mybir.dt.bfloat16
    AF = mybir.ActivationFunctionType

    nc = bacc.Bacc()

    xt = nc.dram_tensor("xt", [D, T], bf16, kind="ExternalInput")
    wq = nc.dram_tensor("wq", [128, NDC, HQ], bf16, kind="ExternalInput")
    wk = nc.dram_tensor("wk", [128, NDC, HD], bf16, kind="ExternalInput")
    wv = nc.dram_tensor("wv", [128, NDC, HD], bf16, kind="ExternalInput")
    wo = nc.dram_tensor("wo", [128, GQ, D], bf16, kind="ExternalInput")
    cosd = nc.dram_tensor("cos", [HD, T], bf16, kind="ExternalInput")
    sind = nc.dram_tensor("sin", [HD, T], bf16, kind="ExternalInput")
    permd = nc.dram_tensor("perm", [128, 128], bf16, kind="ExternalInput")
    trid = nc.dram_tensor("tri", [128, 128], bf16, kind="ExternalInput")
    onesd = nc.dram_tensor("ones", [128, 128], bf16, kind="ExternalInput")
    onesnd = nc.dram_tensor("onesn", [128, 128], bf16, kind="ExternalInput")
    wqcd = nc.dram_tensor("wqc", [HD, 1], f32, kind="ExternalInput")
    wkcd = nc.dram_tensor("wkc", [HD, 1], f32, kind="ExternalInput")
    wqed = nc.dram_tensor("wqe", [HD, 1], f32, kind="ExternalInput")
    wked = nc.dram_tensor("wke", [HD, 1], f32, kind="ExternalInput")
    outd = nc.dram_tensor("out", [T, D], f32, kind="ExternalOutput")

    xtr = xt.rearrange("(c p) t -> p c t", p=128)

    with tile.TileContext(nc) as tc:
        with (
            tc.tile_pool(name="A", bufs=1) as A,
            tc.tile_pool(name="BX", bufs=3) as BX,
            tc.tile_pool(name="TMP", bufs=3) as TMP,
            tc.tile_pool(name="CP", bufs=6) as CP,
            tc.tile_pool(name="CT", bufs=4) as CT,
            tc.tile_pool(name="CO", bufs=8) as CO,
        ):
            # persistent tiles
            QT = A.tile([128, GQ, T], bf16, tag="QT")
            KT = A.tile([128, T], bf16, tag="KT")
            Vn = A.tile([128, NE * 2, HD], bf16, tag="Vn")
            wq_sb = A.tile([128, NDC, HQ], bf16, tag="wq")
            wk_sb = A.tile([128, NDC, HD], bf16, tag="wk")
            wv_sb = A.tile([128, NDC, HD], bf16, tag="wv")
            wo_sb = A.tile([128, GQ, D], bf16, tag="wo")
            cos_sb = A.tile([128, T], bf16, tag="cos")
            sin_sb = A.tile([128, T], bf16, tag="sin")
            perm_sb = A.tile([128, 128], bf16, tag="perm")
            tri_sb = A.tile([128, 128], bf16, tag="tri")
            ones_sb = A.tile([128, 128], bf16, tag="ones")
            onesn_sb = A.tile([128, 128], bf16, tag="onesn")
            wqc = A.tile([128, 1], f32, tag="wqc")
            wkc = A.tile([128, 1], f32, tag="wkc")
            wqe = A.tile([128, 1], f32, tag="wqe")
            wke = A.tile([128, 1], f32, tag="wke")
            OTJ = [A.tile([128, GQ, JW], bf16, tag="OTa", name="OTa"),
                   A.tile([128, GQ, JW], bf16, tag="OTb", name="OTb")]

            # Dummy first activation: a Sqrt, so the act-table pass loads the
            # sqrt set (which also holds square+copy) once at startup instead
            # of loading the square set first and swapping mid-phase-1.
            warm = A.tile([128, 1], f32, tag="warm")
            nc.vector.memset(warm, 1.0)
            nc.scalar.activation(warm, warm, AF.Sqrt)

            # staged preload: wk -> xt0 -> wv -> wq -> tables -> xt1 -> wo
            nc.sync.dma_start(out=wk_sb, in_=wk[:, :, :])
            xt_bufs = {}
            xt_bufs[0] = BX.tile([128, NDC, ET], bf16, tag="xt", name="xt0")
            for g in range(4):
                nc.sync.dma_start(out=xt_bufs[0][:, ts(g, 4), :],
                                  in_=xtr[:, ts(g, 4), 0:ET])
            nc.sync.dma_start(out=wv_sb, in_=wv[:, :, :])
            nc.sync.dma_start(out=wq_sb[:, 0:8, :], in_=wq[:, 0:8, :])
            nc.sync.dma_start(out=wq_sb[:, 8:16, :], in_=wq[:, 8:16, :])
            nc.sync.dma_start(out=cos_sb, in_=cosd[:, :])
            nc.sync.dma_start(out=sin_sb, in_=sind[:, :])
            nc.sync.dma_start(out=perm_sb, in_=permd[:, :])
            nc.sync.dma_start(out=tri_sb, in_=trid[:, :])
            nc.sync.dma_start(out=ones_sb, in_=onesd[:, :])
            nc.sync.dma_start(out=onesn_sb, in_=onesnd[:, :])
            nc.sync.dma_start(out=wqc, in_=wqcd[:, :])
            nc.sync.dma_start(out=wkc, in_=wkcd[:, :])
            nc.sync.dma_start(out=wqe, in_=wqed[:, :])
            nc.sync.dma_start(out=wke, in_=wked[:, :])
            xt_bufs[1] = BX.tile([128, NDC, ET], bf16, tag="xt", name="xt1")
            for g in range(2):
                nc.sync.dma_start(out=xt_bufs[1][:, ts(g, 8), :],
                                  in_=xtr[:, ts(g, 8), ET:2 * ET])
            nc.sync.dma_start(out=wo_sb[:, 0:2, :], in_=wo[:, 0:2, :])
            nc.sync.dma_start(out=wo_sb[:, 2:4, :], in_=wo[:, 2:4, :])

            # ---------------- phase 1: projections ----------------
            with (
                tc.tile_pool(name="PP", bufs=4, space="PSUM") as PP,
                tc.tile_pool(name="PL", bufs=2, space="PSUM") as PL,
                tc.tile_pool(name="PR", bufs=2, space="PSUM") as PR,
            ):
                def normrope(cpsum, wcol, wbias, sl, out_sl):
                    """RMSNorm (partition-dim mean via ones-matmul) + norm
                    weight + RoPE (rotate-half via signed-permutation matmul
                    on PE). Writes bf16 out_sl [128, ET]."""
                    sq = TMP.tile([128, ET], bf16, tag="sq")
                    nc.scalar.activation(sq, cpsum, AF.Square)
                    l2 = PL.tile([128, ET], f32, tag="l2")
                    nc.tensor.matmul(l2, onesn_sb, sq, start=True, stop=True)
                    sv = TMP.tile([128, ET], f32, tag="sv")
                    nc.scalar.activation(sv, l2, AF.Sqrt, scale=wcol, bias=wbias)
                    rc = TMP.tile([128, ET], f32, tag="rc")
                    nc.vector.reciprocal(rc, sv)
                    qn = TMP.tile([128, ET], bf16, tag="qn")
                    nc.vector.tensor_mul(qn, cpsum, rc)
                    qr = PR.tile([128, ET], f32, tag="qr")
                    nc.tensor.matmul(qr, perm_sb, qn, start=True, stop=True)
                    t1 = TMP.tile([128, ET], bf16, tag="t1")
                    nc.vector.tensor_mul(t1, qn, cos_sb[:, sl])
                    t2 = TMP.tile([128, ET], bf16, tag="t2")
                    nc.vector.tensor_mul(t2, qr, sin_sb[:, sl])
                    nc.vector.tensor_add(out_sl, t1, t2)

                for e in range(NE):
                    sl = ts(e, ET)
                    if e + 2 < NE:
                        xt_bufs[e + 2] = BX.tile([128, NDC, ET], bf16,
                                                 tag="xt", name=f"xt{e + 2}")
                        for g in range(2):
                            nc.sync.dma_start(
                                out=xt_bufs[e + 2][:, ts(g, 8), :],
                                in_=xtr[:, ts(g, 8), (e + 2) * ET:(e + 3) * ET])
                    xt_t = xt_bufs.pop(e)
                    # K projection
                    kp = PP.tile([128, ET], f32, tag="pp", name="kp")
                    for c in range(NDC):
                        nc.tensor.matmul(kp, wk_sb[:, c, :], xt_t[:, c, :],
                                         start=(c == 0), stop=(c == NDC - 1))
                    # V natural-layout projections
                    vps = []
                    for i in range(2):
                        vp = PP.tile([128, HD], f32, tag="pp", name=f"vp{i}")
                        for c in range(NDC):
                            nc.tensor.matmul(vp, xt_t[:, c, ts(i, 128)],
                                             wv_sb[:, c, :],
                                             start=(c == 0), stop=(c == NDC - 1))
                        vps.append(vp)
                    normrope(kp, wkc, wke, sl, KT[:, sl])
                    for i in range(2):
                        nc.scalar.activation(Vn[:, 2 * e + i, :], vps[i],
                                             AF.Copy)
                    # Q heads
                    for h in range(GQ):
                        qp = PP.tile([128, ET], f32, tag="pp", name=f"qp{h}")
                        for c in range(NDC):
                            nc.tensor.matmul(qp, wq_sb[:, c, ts(h, 128)],
                                             xt_t[:, c, :],
                                             start=(c == 0), stop=(c == NDC - 1))
                        normrope(qp, wqc, wqe, sl, QT[:, h, sl])

            # ---------------- phase 2: attention + out projection ---------
            with (
                tc.tile_pool(name="PS2", bufs=3, space="PSUM") as PS2,
                tc.tile_pool(name="PLP", bufs=1, space="PSUM") as PLP,
                tc.tile_pool(name="POP", bufs=1, space="PSUM") as POP,
                tc.tile_pool(name="PS3", bufs=3, space="PSUM") as PS3,
            ):
                def outproj_chunk(Jm, c, spread=False):
                    """Output projection: column chunk c (of 4) for the four
                    128-row q tiles of block Jm; streams each [128,512] chunk
                    straight out. spread=True rotates oup allocations through
                    the idle lp/op banks too (used for the final block, which
                    has no attention work to hide the bank latency)."""
                    for qt in range(4):
                        qtg = 4 * Jm + qt
                        if spread and qt == 1:
                            oup = PLP.tile([128, 512], f32, tag="lp")
                        elif spread and qt == 2:
                            oup = POP.tile([128, 512], f32, tag="op")
                        else:
                            oup = PS3.tile([128, 512], f32, tag="oup")
                        for hc in range(GQ):
                            nc.tensor.matmul(oup, OTJ[Jm % 2][:, hc, ts(qt, 128)],
                                             wo_sb[:, hc, ts(c, 512)],
                                             start=(hc == 0), stop=(hc == GQ - 1))
                        oc = CO.tile([128, 512], f32, tag="oc")
                        nc.scalar.activation(oc, oup, AF.Copy)
                        nc.sync.dma_start(
                            out=outd[qtg * 128:(qtg + 1) * 128,
                                     c * 512:(c + 1) * 512],
                            in_=oc)

                for J in range(NJ):
                    nkb = 4 * J + 4
                    for h in range(GQ):
                        lp = PLP.tile([128, JW], f32, tag="lp")
                        op = POP.tile([128, JW], f32, tag="op")
                        for kb in range(nkb):
                            r = kb - 4 * J  # >= 0 on the diagonal blocks
                            c0 = 128 * r if r > 0 else 0
                            sp = PS2.tile([128, JW], f32, tag="s")
                            nc.tensor.matmul(sp[:, c0:JW], KT[:, ts(kb, 128)],
                                             QT[:, h, J * JW + c0:(J + 1) * JW],
                                             start=True, stop=True,
                                             skip_group_check=True)
                            P = CP.tile([128, JW], bf16, tag="p")
                            nc.scalar.activation(P[:, c0:JW], sp[:, c0:JW],
                                                 AF.Exp, scale=SCALE)
                            if r >= 0:
                                nc.vector.tensor_mul(
                                    P[:, 128 * r:128 * r + 128],
                                    P[:, 128 * r:128 * r + 128], tri_sb)
                            nc.tensor.matmul(lp[:, c0:JW], ones_sb, P[:, c0:JW],
                                             start=(kb == 0), stop=(kb == nkb - 1),
                                             skip_group_check=True)
                            nc.tensor.matmul(op[:, c0:JW], Vn[:, kb, :],
                                             P[:, c0:JW],
                                             start=(kb == 0), stop=(kb == nkb - 1),
                                             skip_group_check=True)
                        rc2 = CT.tile([128, JW], f32, tag="rc2")
                        nc.vector.reciprocal(rc2, lp)
                        nc.vector.tensor_mul(OTJ[J % 2][:, h, :], op, rc2)
                        if J > 0:
                            outproj_chunk(J - 1, h)
                for c in range(4):
                    outproj_chunk(NJ - 1, c)

    nc.finalize()
    return nc


def _host_consts():
    import ml_dtypes
    bf16 = ml_dtypes.bfloat16
    inv = 1.0 / (ROPE_BASE ** (np.arange(0, HD, 2, dtype=np.float64) / HD))
    freqs = np.outer(np.arange(T, dtype=np.float64), inv)
    emb = np.concatenate([freqs, freqs], axis=-1)          # [T, HD]
    cosT = np.ascontiguousarray(np.cos(emb).T.astype(bf16))  # [HD, T]
    sinT = np.ascontiguousarray(np.sin(emb).T.astype(bf16))
    # signed permutation for rotate-half: qr[m] = -qn[m+64] (m<64), qn[m-64]
    perm = np.zeros((128, 128), np.float32)
    for m in range(64):
        perm[m + 64, m] = -1.0
        perm[m, m + 64] = 1.0
    perm = perm.astype(bf16)
    k = np.arange(128)[:, None]
    q = np.arange(128)[None, :]
    tri = (k <= q).astype(bf16)
    ones = np.ones((128, 128), bf16)
    onesn = np.full((128, 128), 1.0 / HD, bf16)
    return cosT, sinT, perm, tri, ones, onesn


def kernel(x, Wq, Wk, Wv, Wo, q_norm_w, k_norm_w):
    import ml_dtypes
    from concourse.bass_utils import run_bass_kernel_spmd
    bf16 = ml_dtypes.bfloat16

    if "nc" not in _cached:
        _cached["nc"] = _build_program()
        _cached["consts"] = _host_consts()
    nc = _cached["nc"]
    cosT, sinT, perm, tri, ones, onesn = _cached["consts"]

    x = np.asarray(x, np.float32)
    Wq = np.asarray(Wq, np.float32)
    Wk = np.asarray(Wk, np.float32)
    Wv = np.asarray(Wv, np.float32)
    Wo = np.asarray(Wo, np.float32)
    qwf = np.asarray(q_norm_w, np.float64).reshape(HD, 1)
    kwf = np.asarray(k_norm_w, np.float64).reshape(HD, 1)
    qw = np.ascontiguousarray((1.0 / qwf ** 2).astype(np.float32))
    kw = np.ascontiguousarray((1.0 / kwf ** 2).astype(np.float32))
    qwe = np.ascontiguousarray((EPS / qwf ** 2).astype(np.float32))
    kwe = np.ascontiguousarray((EPS / kwf ** 2).astype(np.float32))

    xTb = [np.ascontiguousarray(x[b].T.astype(bf16)) for b in range(B)]
    in_maps = []
    for core in range(8):
        b, kv = divmod(core, NKV)
        wq_p = np.ascontiguousarray(
            Wq[:, kv * HQ:(kv + 1) * HQ].reshape(NDC, 128, HQ)
            .transpose(1, 0, 2).astype(bf16))
        wk_p = np.ascontiguousarray(
            Wk[:, kv * HD:(kv + 1) * HD].reshape(NDC, 128, HD)
            .transpose(1, 0, 2).astype(bf16))
        wv_p = np.ascontiguousarray(
            Wv[:, kv * HD:(kv + 1) * HD].reshape(NDC, 128, HD)
            .transpose(1, 0, 2).astype(bf16))
        wo_p = np.ascontiguousarray(
            Wo[kv * HQ:(kv + 1) * HQ, :].reshape(GQ, 128, D)
            .transpose(1, 0, 2).astype(bf16))
        in_maps.append({
            "xt": xTb[b],
            "wq": wq_p, "wk": wk_p, "wv": wv_p, "wo": wo_p,
            "cos": cosT, "sin": sinT,
            "perm": perm, "tri": tri, "ones": ones, "onesn": onesn,
            "wqc": qw, "wkc": kw, "wqe": qwe, "wke": kwe,
        })
    res = run_bass_kernel_spmd(nc, in_maps, list(range(8)))
    out = np.zeros((B, T, D), np.float64)
    for core in range(8):
        b = core // NKV
        out[b] += res.results[core]["out"].astype(np.float64)
    return out.astype(np.float32)


# revision 23
# speedup vs baseline: 1.0119x; 1.0062x over previous
"""GQA kernel for Trainium2, 8 NeuronCores.

Sharding: core c = b*4 + kv  (b in {0,1} data-parallel over batch,
kv in {0..3} tensor-parallel over the 4 KV head groups; each core owns
4 Q heads + 1 KV head). Each core computes a partial output
x[b] @ Wq[:,kv] -> attention -> @ Wo[kv rows]; host sums the 4 partials
per batch (the row-sharded-Wo all-reduce).

v3 (all matmul operands bf16; PSUM accumulation f32; one PSUM
accumulation group per bank - groups may NOT share banks):

Phase 1, per 256-row eighth: K projection, V projection in natural
layout (bf16 N=128 matmuls - no PE transposes), 4 Q projections; each
followed by in-layout RMSNorm+RoPE where rotate-half is a
signed-permutation matmul on PE (no SBUF-SBUF DMAs). xt streamed in
bf16 with depth-2 prefetch; weights staged wk->xt0->wv->wq->tables->wo.

Phase 2, per 512-col q block J: S^T[k,q] = K Q^T per 128-key block,
exp on Act, softmax denominator via ones-matmul accumulated in PSUM,
O^T accumulated over key blocks; diagonal key blocks column-shrunk
(bf16 matmuls have no minimum-N penalty). Output projection of block
J-1 is interleaved between heads of block J, streamed out in
[128,512] chunks.
"""

import numpy as np

B, T, D = 2, 2048, 2048
NH, NKV, HD = 16, 4, 128
GQ = NH // NKV            # 4 q heads per kv head
HQ = GQ * HD              # 512 q-dim per core
ROPE_BASE = 500000.0
EPS = 1e-5
SCALE = 1.0 / np.sqrt(HD)
NE = 8                    # T eighths (phase 1)
ET = T // NE              # 256
NDC = D // 128            # 16 contraction chunks
NJ = 4                    # phase-2 q blocks
JW = T // NJ              # 512

_cached = {}


def _build_program():
    import concourse.bacc as bacc
    import concourse.mybir as mybir
    from concourse import tile
    from concourse.bass import ts

    f32 = mybir.dt.float32
    bf16 = mybir.dt.bfloat16
    AF = mybir.ActivationFunctionType

    nc = bacc.Bacc()

    xt = nc.dram_tensor("xt", [D, T], bf16, kind="ExternalInput")
    wq = nc.dram_tensor("wq", [128, NDC, HQ], bf16, kind="ExternalInput")
    wk = nc.dram_tensor("wk", [128, NDC, HD], bf16, kind="ExternalInput")
    wv = nc.dram_tensor("wv", [128, NDC, HD], bf16, kind="ExternalInput")
    wo = nc.dram_tensor("wo", [128, GQ, D], bf16, kind="ExternalInput")
    cosd = nc.dram_tensor("cos", [HD, T], bf16, kind="ExternalInput")
    sind = nc.dram_tensor("sin", [HD, T], bf16, kind="ExternalInput")
    permd = nc.dram_tensor("perm", [128, 128], bf16, kind="ExternalInput")
    trid = nc.dram_tensor("tri", [128, 128], bf16, kind="ExternalInput")
    onesd = nc.dram_tensor("ones", [128, 128], bf16, kind="ExternalInput")
    onesnd = nc.dram_tensor("onesn", [128, 128], bf16, kind="ExternalInput")
    wqcd = nc.dram_tensor("wqc", [HD, 1], f32, kind="ExternalInput")
    wkcd = nc.dram_tensor("wkc", [HD, 1], f32, kind="ExternalInput")
    wqed = nc.dram_tensor("wqe", [HD, 1], f32, kind="ExternalInput")
    wked = nc.dram_tensor("wke", [HD, 1], f32, kind="ExternalInput")
    outd = nc.dram_tensor("out", [T, D], f32, kind="ExternalOutput")

    xtr = xt.rearrange("(c p) t -> p c t", p=128)

    with tile.TileContext(nc) as tc:
        with (
            tc.tile_pool(name="A", bufs=1) as A,
            tc.tile_pool(name="BX", bufs=3) as BX,
            tc.tile_pool(name="TMP", bufs=3) as TMP,
            tc.tile_pool(name="CP", bufs=6) as CP,
            tc.tile_pool(name="CT", bufs=4) as CT,
            tc.tile_pool(name="CO", bufs=12) as CO,
        ):
            # persistent tiles
            QT = A.tile([128, GQ, T], bf16, tag="QT")
            KT = A.tile([128, T], bf16, tag="KT")
            Vn = A.tile([128, NE * 2, HD], bf16, tag="Vn")
            wq_sb = A.tile([128, NDC, HQ], bf16, tag="wq")
            wk_sb = A.tile([128, NDC, HD], bf16, tag="wk")
            wv_sb = A.tile([128, NDC, HD], bf16, tag="wv")
            wo_sb = A.tile([128, GQ, D], bf16, tag="wo")
            cos_sb = A.tile([128, T], bf16, tag="cos")
            sin_sb = A.tile([128, T], bf16, tag="sin")
            perm_sb = A.tile([128, 128], bf16, tag="perm")
            tri_sb = A.tile([128, 128], bf16, tag="tri")
            ones_sb = A.tile([128, 128], bf16, tag="ones")
            onesn_sb = A.tile([128, 128], bf16, tag="onesn")
            wqc = A.tile([128, 1], f32, tag="wqc")
            wkc = A.tile([128, 1], f32, tag="wkc")
            wqe = A.tile([128, 1], f32, tag="wqe")
            wke = A.tile([128, 1], f32, tag="wke")
            OTJ = [A.tile([128, GQ, JW], bf16, tag="OTa", name="OTa"),
                   A.tile([128, GQ, JW], bf16, tag="OTb", name="OTb")]

            # Dummy first activation: a Sqrt, so the act-table pass loads the
            # sqrt set (which also holds square+copy) once at startup instead
            # of loading the square set first and swapping mid-phase-1.
            warm = A.tile([128, 1], f32, tag="warm")
            nc.vector.memset(warm, 1.0)
            nc.scalar.activation(warm, warm, AF.Sqrt)

            # staged preload: wk -> xt0 -> wv -> wq -> tables -> xt1 -> wo
            # (wk and xt0 interleaved in matching halves so the first K
            # matmuls can start as early as possible)
            nc.sync.dma_start(out=wk_sb[:, 0:8, :], in_=wk[:, 0:8, :])
            xt_bufs = {}
            xt_bufs[0] = BX.tile([128, NDC, ET], bf16, tag="xt", name="xt0")
            for g in range(2):
                nc.sync.dma_start(out=xt_bufs[0][:, ts(g, 4), :],
                                  in_=xtr[:, ts(g, 4), 0:ET])
            nc.sync.dma_start(out=wk_sb[:, 8:16, :], in_=wk[:, 8:16, :])
            for g in range(2, 4):
                nc.sync.dma_start(out=xt_bufs[0][:, ts(g, 4), :],
                                  in_=xtr[:, ts(g, 4), 0:ET])
            nc.sync.dma_start(out=wv_sb, in_=wv[:, :, :])
            nc.sync.dma_start(out=wq_sb[:, 0:8, :], in_=wq[:, 0:8, :])
            nc.sync.dma_start(out=wq_sb[:, 8:16, :], in_=wq[:, 8:16, :])
            nc.sync.dma_start(out=cos_sb, in_=cosd[:, :])
            nc.sync.dma_start(out=sin_sb, in_=sind[:, :])
            nc.sync.dma_start(out=perm_sb, in_=permd[:, :])
            nc.sync.dma_start(out=tri_sb, in_=trid[:, :])
            nc.sync.dma_start(out=ones_sb, in_=onesd[:, :])
            nc.sync.dma_start(out=onesn_sb, in_=onesnd[:, :])
            nc.sync.dma_start(out=wqc, in_=wqcd[:, :])
            nc.sync.dma_start(out=wkc, in_=wkcd[:, :])
            nc.sync.dma_start(out=wqe, in_=wqed[:, :])
            nc.sync.dma_start(out=wke, in_=wked[:, :])
            xt_bufs[1] = BX.tile([128, NDC, ET], bf16, tag="xt", name="xt1")
            for g in range(2):
                nc.sync.dma_start(out=xt_bufs[1][:, ts(g, 8), :],
                                  in_=xtr[:, ts(g, 8), ET:2 * ET])
            nc.sync.dma_start(out=wo_sb[:, 0:2, :], in_=wo[:, 0:2, :])
            nc.sync.dma_start(out=wo_sb[:, 2:4, :], in_=wo[:, 2:4, :])

            # ---------------- phase 1: projections ----------------
            with (
                tc.tile_pool(name="PP", bufs=4, space="PSUM") as PP,
                tc.tile_pool(name="PL", bufs=2, space="PSUM") as PL,
                tc.tile_pool(name="PR", bufs=2, space="PSUM") as PR,
            ):
                def normrope(cpsum, wcol, wbias, sl, out_sl):
                    """RMSNorm (partition-dim mean via ones-matmul) + norm
                    weight + RoPE (rotate-half via signed-permutation matmul
                    on PE). Writes bf16 out_sl [128, ET]."""
                    sq = TMP.tile([128, ET], bf16, tag="sq")
                    nc.scalar.activation(sq, cpsum, AF.Square)
                    l2 = PL.tile([128, ET], f32, tag="l2")
                    nc.tensor.matmul(l2, onesn_sb, sq, start=True, stop=True)
                    sv = TMP.tile([128, ET], f32, tag="sv")
                    nc.scalar.activation(sv, l2, AF.Sqrt, scale=wcol, bias=wbias)
                    rc = TMP.tile([128, ET], f32, tag="rc")
                    nc.vector.reciprocal(rc, sv)
                    qn = TMP.tile([128, ET], bf16, tag="qn")
                    nc.vector.tensor_mul(qn, cpsum, rc)
                    qr = PR.tile([128, ET], f32, tag="qr")
                    nc.tensor.matmul(qr, perm_sb, qn, start=True, stop=True)
                    t1 = TMP.tile([128, ET], bf16, tag="t1")
                    nc.vector.tensor_mul(t1, qn, cos_sb[:, sl])
                    t2 = TMP.tile([128, ET], bf16, tag="t2")
                    nc.vector.tensor_mul(t2, qr, sin_sb[:, sl])
                    nc.vector.tensor_add(out_sl, t1, t2)

                for e in range(NE):
                    sl = ts(e, ET)
                    if e + 2 < NE:
                        xt_bufs[e + 2] = BX.tile([128, NDC, ET], bf16,
                                                 tag="xt", name=f"xt{e + 2}")
                        for g in range(2):
                            nc.sync.dma_start(
                                out=xt_bufs[e + 2][:, ts(g, 8), :],
                                in_=xtr[:, ts(g, 8), (e + 2) * ET:(e + 3) * ET])
                    xt_t = xt_bufs.pop(e)
                    # K projection
                    kp = PP.tile([128, ET], f32, tag="pp", name="kp")
                    for c in range(NDC):
                        nc.tensor.matmul(kp, wk_sb[:, c, :], xt_t[:, c, :],
                                         start=(c == 0), stop=(c == NDC - 1))

                    def vproj():
                        for i in range(2):
                            vp = PP.tile([128, HD], f32, tag="pp",
                                         name=f"vp{i}")
                            for c in range(NDC):
                                nc.tensor.matmul(vp, xt_t[:, c, ts(i, 128)],
                                                 wv_sb[:, c, :],
                                                 start=(c == 0),
                                                 stop=(c == NDC - 1))
                            nc.scalar.activation(Vn[:, 2 * e + i, :], vp,
                                                 AF.Copy)

                    # V projections right after K, except on the last eighth
                    # where they go last: PE-only work that covers the final
                    # rope chain's DVE drain at the phase boundary.
                    if e < NE - 1:
                        vproj()
                    normrope(kp, wkc, wke, sl, KT[:, sl])
                    # Q heads
                    for h in range(GQ):
                        qp = PP.tile([128, ET], f32, tag="pp", name=f"qp{h}")
                        for c in range(NDC):
                            nc.tensor.matmul(qp, wq_sb[:, c, ts(h, 128)],
                                             xt_t[:, c, :],
                                             start=(c == 0), stop=(c == NDC - 1))
                        normrope(qp, wqc, wqe, sl, QT[:, h, sl])
                    if e == NE - 1:
                        vproj()

            # ---------------- phase 2: attention + out projection ---------
            with (
                tc.tile_pool(name="PS2", bufs=3, space="PSUM") as PS2,
                tc.tile_pool(name="PLP", bufs=1, space="PSUM") as PLP,
                tc.tile_pool(name="POP", bufs=1, space="PSUM") as POP,
                tc.tile_pool(name="PS3", bufs=3, space="PSUM") as PS3,
            ):
                def outproj_chunk(Jm, c, spread=False):
                    """Output projection: column chunk c (of 4) for the four
                    128-row q tiles of block Jm; streams each [128,512] chunk
                    straight out. spread=True rotates oup allocations through
                    the idle lp/op banks too (used for the final block, which
                    has no attention work to hide the bank latency)."""
                    for qt in range(4):
                        qtg = 4 * Jm + qt
                        if spread and qt == 1:
                            oup = PLP.tile([128, 512], f32, tag="lp")
                        elif spread and qt == 2:
                            oup = POP.tile([128, 512], f32, tag="op")
                        else:
                            oup = PS3.tile([128, 512], f32, tag="oup")
                        for hc in range(GQ):
                            nc.tensor.matmul(oup, OTJ[Jm % 2][:, hc, ts(qt, 128)],
                                             wo_sb[:, hc, ts(c, 512)],
                                             start=(hc == 0), stop=(hc == GQ - 1))
                        oc = CO.tile([128, 512], f32, tag="oc")
                        nc.scalar.activation(oc, oup, AF.Copy)
                        nc.sync.dma_start(
                            out=outd[qtg * 128:(qtg + 1) * 128,
                                     c * 512:(c + 1) * 512],
                            in_=oc)

                for J in range(NJ):
                    nkb = 4 * J + 4
                    for h in range(GQ):
                        lp = PLP.tile([128, JW], f32, tag="lp")
                        op = POP.tile([128, JW], f32, tag="op")
                        for kb in range(nkb):
                            r = kb - 4 * J  # >= 0 on the diagonal blocks
                            c0 = 128 * r if r > 0 else 0
                            sp = PS2.tile([128, JW], f32, tag="s")
                            nc.tensor.matmul(sp[:, c0:JW], KT[:, ts(kb, 128)],
                                             QT[:, h, J * JW + c0:(J + 1) * JW],
                                             start=True, stop=True,
                                             skip_group_check=True)
                            P = CP.tile([128, JW], bf16, tag="p")
                            nc.scalar.activation(P[:, c0:JW], sp[:, c0:JW],
                                                 AF.Exp, scale=SCALE)
                            if r >= 0:
                                nc.vector.tensor_mul(
                                    P[:, 128 * r:128 * r + 128],
                                    P[:, 128 * r:128 * r + 128], tri_sb)
                            nc.tensor.matmul(lp[:, c0:JW], ones_sb, P[:, c0:JW],
                                             start=(kb == 0), stop=(kb == nkb - 1),
                                             skip_group_check=True)
                            nc.tensor.matmul(op[:, c0:JW], Vn[:, kb, :],
                                             P[:, c0:JW],
                                             start=(kb == 0), stop=(kb == nkb - 1),
                                             skip_group_check=True)
                        rc2 = CT.tile([128, JW], f32, tag="rc2")
                        nc.vector.reciprocal(rc2, lp)
                        nc.vector.tensor_mul(OTJ[J % 2][:, h, :], op, rc2)
                        if J > 0:
                            outproj_chunk(J - 1, h)
                for c in range(4):
                    outproj_chunk(NJ - 1, c)

    nc.finalize()
    return nc


def _host_consts():
    import ml_dtypes
    bf16 = ml_dtypes.bfloat16
    inv = 1.0 / (ROPE_BASE ** (np.arange(0, HD, 2, dtype=np.float64) / HD))
    freqs = np.outer(np.arange(T, dtype=np.float64), inv)
    emb = np.concatenate([freqs, freqs], axis=-1)          # [T, HD]
    cosT = np.ascontiguousarray(np.cos(emb).T.astype(bf16))  # [HD, T]
    sinT = np.ascontiguousarray(np.sin(emb).T.astype(bf16))
    # signed permutation for rotate-half: qr[m] = -qn[m+64] (m<64), qn[m-64]
    perm = np.zeros((128, 128), np.float32)
    for m in range(64):
        perm[m + 64, m] = -1.0
        perm[m, m + 64] = 1.0
    perm = perm.astype(bf16)
    k = np.arange(128)[:, None]
    q = np.arange(128)[None, :]
    tri = (k <= q).astype(bf16)
    ones = np.ones((128, 128), bf16)
    onesn = np.full((128, 128), 1.0 / HD, bf16)
    return cosT, sinT, perm, tri, ones, onesn


def kernel(x, Wq, Wk, Wv, Wo, q_norm_w, k_norm_w):
    import ml_dtypes
    from concourse.bass_utils import run_bass_kernel_spmd
    bf16 = ml_dtypes.bfloat16

    if "nc" not in _cached:
        _cached["nc"] = _build_program()
        _cached["consts"] = _host_consts()
    nc = _cached["nc"]
    cosT, sinT, perm, tri, ones, onesn = _cached["consts"]

    x = np.asarray(x, np.float32)
    Wq = np.asarray(Wq, np.float32)
    Wk = np.asarray(Wk, np.float32)
    Wv = np.asarray(Wv, np.float32)
    Wo = np.asarray(Wo, np.float32)
    qwf = np.asarray(q_norm_w, np.float64).reshape(HD, 1)
    kwf = np.asarray(k_norm_w, np.float64).reshape(HD, 1)
    qw = np.ascontiguousarray((1.0 / qwf ** 2).astype(np.float32))
    kw = np.ascontiguousarray((1.0 / kwf ** 2).astype(np.float32))
    qwe = np.ascontiguousarray((EPS / qwf ** 2).astype(np.float32))
    kwe = np.ascontiguousarray((EPS / kwf ** 2).astype(np.float32))

    xTb = [np.ascontiguousarray(x[b].T.astype(bf16)) for b in range(B)]
    in_maps = []
    for core in range(8):
        b, kv = divmod(core, NKV)
        wq_p = np.ascontiguousarray(
            Wq[:, kv * HQ:(kv + 1) * HQ].reshape(NDC, 128, HQ)
            .transpose(1, 0, 2).astype(bf16))
        wk_p = np.ascontiguousarray(
            Wk[:, kv * HD:(kv + 1) * HD].reshape(NDC, 128, HD)
            .transpose(1, 0, 2).astype(bf16))
        wv_p = np.ascontiguousarray(
            Wv[:, kv * HD:(kv + 1) * HD].reshape(NDC, 128, HD)
            .transpose(1, 0, 2).astype(bf16))
        wo_p = np.ascontiguousarray(
            Wo[kv * HQ:(kv + 1) * HQ, :].reshape(GQ, 128, D)
            .transpose(1, 0, 2).astype(bf16))
        in_maps.append({
            "xt": xTb[b],
            "wq": wq_p, "wk": wk_p, "wv": wv_p, "wo": wo_p,
            "cos": cosT, "sin": sinT,
            "perm": perm, "tri": tri, "ones": ones, "onesn": onesn,
            "wqc": qw, "wkc": kw, "wqe": qwe, "wke": kwe,
        })
    res = run_bass_kernel_spmd(nc, in_maps, list(range(8)))
    out = np.zeros((B, T, D), np.float64)
    for core in range(8):
        b = core // NKV
        out[b] += res.results[core]["out"].astype(np.float64)
    return out.astype(np.float32)


# revision 24
# speedup vs baseline: 1.0119x; 1.0000x over previous
"""GQA kernel for Trainium2, 8 NeuronCores.

Sharding: core c = b*4 + kv  (b in {0,1} data-parallel over batch,
kv in {0..3} tensor-parallel over the 4 KV head groups; each core owns
4 Q heads + 1 KV head). Each core computes a partial output
x[b] @ Wq[:,kv] -> attention -> @ Wo[kv rows]; host sums the 4 partials
per batch (the row-sharded-Wo all-reduce).

v3 (all matmul operands bf16; PSUM accumulation f32; one PSUM
accumulation group per bank - groups may NOT share banks):

Phase 1, per 256-row eighth: K projection, V projection in natural
layout (bf16 N=128 matmuls - no PE transposes), 4 Q projections; each
followed by in-layout RMSNorm+RoPE where rotate-half is a
signed-permutation matmul on PE (no SBUF-SBUF DMAs). xt streamed in
bf16 with depth-2 prefetch; weights staged wk->xt0->wv->wq->tables->wo.

Phase 2, per 512-col q block J: S^T[k,q] = K Q^T per 128-key block,
exp on Act, softmax denominator via ones-matmul accumulated in PSUM,
O^T accumulated over key blocks; diagonal key blocks column-shrunk
(bf16 matmuls have no minimum-N penalty). Output projection of block
J-1 is interleaved between heads of block J, streamed out in
[128,512] chunks.
"""

import numpy as np

B, T, D = 2, 2048, 2048
NH, NKV, HD = 16, 4, 128
GQ = NH // NKV            # 4 q heads per kv head
HQ = GQ * HD              # 512 q-dim per core
ROPE_BASE = 500000.0
EPS = 1e-5
SCALE = 1.0 / np.sqrt(HD)
NE = 8                    # T eighths (phase 1)
ET = T // NE              # 256
NDC = D // 128            # 16 contraction chunks
NJ = 4                    # phase-2 q blocks
JW = T // NJ              # 512

_cached = {}


def _build_program():
    import concourse.bacc as bacc
    import concourse.mybir as mybir
    from concourse import tile
    from concourse.bass import ts

    f32 = mybir.dt.float32
    bf16 = mybir.dt.bfloat16
    AF = mybir.ActivationFunctionType

    nc = bacc.Bacc()

    xt = nc.dram_tensor("xt", [D, T], bf16, kind="ExternalInput")
    wq = nc.dram_tensor("wq", [128, NDC, HQ], bf16, kind="ExternalInput")
    wk = nc.dram_tensor("wk", [128, NDC, HD], bf16, kind="ExternalInput")
    wv = nc.dram_tensor("wv", [128, NDC, HD], bf16, kind="ExternalInput")
    wo = nc.dram_tensor("wo", [128, GQ, D], bf16, kind="ExternalInput")
    cosd = nc.dram_tensor("cos", [HD, T], bf16, kind="ExternalInput")
    sind = nc.dram_tensor("sin", [HD, T], bf16, kind="ExternalInput")
    permd = nc.dram_tensor("perm", [128, 128], bf16, kind="ExternalInput")
    trid = nc.dram_tensor("tri", [128, 128], bf16, kind="ExternalInput")
    onesd = nc.dram_tensor("ones", [128, 128], bf16, kind="ExternalInput")
    onesnd = nc.dram_tensor("onesn", [128, 128], bf16, kind="ExternalInput")
    wqcd = nc.dram_tensor("wqc", [HD, 1], f32, kind="ExternalInput")
    wkcd = nc.dram_tensor("wkc", [HD, 1], f32, kind="ExternalInput")
    wqed = nc.dram_tensor("wqe", [HD, 1], f32, kind="ExternalInput")
    wked = nc.dram_tensor("wke", [HD, 1], f32, kind="ExternalInput")
    outd = nc.dram_tensor("out", [T, D], f32, kind="ExternalOutput")

    xtr = xt.rearrange("(c p) t -> p c t", p=128)

    with tile.TileContext(nc) as tc:
        with (
            tc.tile_pool(name="A", bufs=1) as A,
            tc.tile_pool(name="BX", bufs=3) as BX,
            tc.tile_pool(name="TMP", bufs=4) as TMP,
            tc.tile_pool(name="CP", bufs=8) as CP,
            tc.tile_pool(name="CT", bufs=4) as CT,
            tc.tile_pool(name="CO", bufs=12) as CO,
        ):
            # persistent tiles
            QT = A.tile([128, GQ, T], bf16, tag="QT")
            KT = A.tile([128, T], bf16, tag="KT")
            Vn = A.tile([128, NE * 2, HD], bf16, tag="Vn")
            wq_sb = A.tile([128, NDC, HQ], bf16, tag="wq")
            wk_sb = A.tile([128, NDC, HD], bf16, tag="wk")
            wv_sb = A.tile([128, NDC, HD], bf16, tag="wv")
            wo_sb = A.tile([128, GQ, D], bf16, tag="wo")
            cos_sb = A.tile([128, T], bf16, tag="cos")
            sin_sb = A.tile([128, T], bf16, tag="sin")
            perm_sb = A.tile([128, 128], bf16, tag="perm")
            tri_sb = A.tile([128, 128], bf16, tag="tri")
            ones_sb = A.tile([128, 128], bf16, tag="ones")
            onesn_sb = A.tile([128, 128], bf16, tag="onesn")
            wqc = A.tile([128, 1], f32, tag="wqc")
            wkc = A.tile([128, 1], f32, tag="wkc")
            wqe = A.tile([128, 1], f32, tag="wqe")
            wke = A.tile([128, 1], f32, tag="wke")
            OTJ = [A.tile([128, GQ, JW], bf16, tag="OTa", name="OTa"),
                   A.tile([128, GQ, JW], bf16, tag="OTb", name="OTb")]

            # Dummy first activation: a Sqrt, so the act-table pass loads the
            # sqrt set (which also holds square+copy) once at startup instead
            # of loading the square set first and swapping mid-phase-1.
            warm = A.tile([128, 1], f32, tag="warm")
            nc.vector.memset(warm, 1.0)
            nc.scalar.activation(warm, warm, AF.Sqrt)

            # staged preload: wk -> xt0 -> wv -> wq -> tables -> xt1 -> wo
            # (wk and xt0 interleaved in matching halves so the first K
            # matmuls can start as early as possible)
            nc.sync.dma_start(out=wk_sb[:, 0:8, :], in_=wk[:, 0:8, :])
            xt_bufs = {}
            xt_bufs[0] = BX.tile([128, NDC, ET], bf16, tag="xt", name="xt0")
            for g in range(2):
                nc.sync.dma_start(out=xt_bufs[0][:, ts(g, 4), :],
                                  in_=xtr[:, ts(g, 4), 0:ET])
            nc.sync.dma_start(out=wk_sb[:, 8:16, :], in_=wk[:, 8:16, :])
            for g in range(2, 4):
                nc.sync.dma_start(out=xt_bufs[0][:, ts(g, 4), :],
                                  in_=xtr[:, ts(g, 4), 0:ET])
            nc.sync.dma_start(out=wv_sb, in_=wv[:, :, :])
            nc.sync.dma_start(out=wq_sb[:, 0:8, :], in_=wq[:, 0:8, :])
            nc.sync.dma_start(out=wq_sb[:, 8:16, :], in_=wq[:, 8:16, :])
            nc.sync.dma_start(out=cos_sb, in_=cosd[:, :])
            nc.sync.dma_start(out=sin_sb, in_=sind[:, :])
            nc.sync.dma_start(out=perm_sb, in_=permd[:, :])
            nc.sync.dma_start(out=tri_sb, in_=trid[:, :])
            nc.sync.dma_start(out=ones_sb, in_=onesd[:, :])
            nc.sync.dma_start(out=onesn_sb, in_=onesnd[:, :])
            nc.sync.dma_start(out=wqc, in_=wqcd[:, :])
            nc.sync.dma_start(out=wkc, in_=wkcd[:, :])
            nc.sync.dma_start(out=wqe, in_=wqed[:, :])
            nc.sync.dma_start(out=wke, in_=wked[:, :])
            xt_bufs[1] = BX.tile([128, NDC, ET], bf16, tag="xt", name="xt1")
            for g in range(2):
                nc.sync.dma_start(out=xt_bufs[1][:, ts(g, 8), :],
                                  in_=xtr[:, ts(g, 8), ET:2 * ET])
            nc.sync.dma_start(out=wo_sb[:, 0:2, :], in_=wo[:, 0:2, :])
            nc.sync.dma_start(out=wo_sb[:, 2:4, :], in_=wo[:, 2:4, :])

            # ---------------- phase 1: projections ----------------
            with (
                tc.tile_pool(name="PP", bufs=4, space="PSUM") as PP,
                tc.tile_pool(name="PL", bufs=2, space="PSUM") as PL,
                tc.tile_pool(name="PR", bufs=2, space="PSUM") as PR,
            ):
                def normrope(cpsum, wcol, wbias, sl, out_sl):
                    """RMSNorm (partition-dim mean via ones-matmul) + norm
                    weight + RoPE (rotate-half via signed-permutation matmul
                    on PE). Writes bf16 out_sl [128, ET]."""
                    sq = TMP.tile([128, ET], bf16, tag="sq")
                    nc.scalar.activation(sq, cpsum, AF.Square)
                    l2 = PL.tile([128, ET], f32, tag="l2")
                    nc.tensor.matmul(l2, onesn_sb, sq, start=True, stop=True)
                    sv = TMP.tile([128, ET], f32, tag="sv")
                    nc.scalar.activation(sv, l2, AF.Sqrt, scale=wcol, bias=wbias)
                    rc = TMP.tile([128, ET], f32, tag="rc")
                    nc.vector.reciprocal(rc, sv)
                    qn = TMP.tile([128, ET], bf16, tag="qn")
                    nc.vector.tensor_mul(qn, cpsum, rc)
                    qr = PR.tile([128, ET], f32, tag="qr")
                    nc.tensor.matmul(qr, perm_sb, qn, start=True, stop=True)
                    t1 = TMP.tile([128, ET], bf16, tag="t1")
                    nc.vector.tensor_mul(t1, qn, cos_sb[:, sl])
                    t2 = TMP.tile([128, ET], bf16, tag="t2")
                    nc.vector.tensor_mul(t2, qr, sin_sb[:, sl])
                    nc.vector.tensor_add(out_sl, t1, t2)

                for e in range(NE):
                    sl = ts(e, ET)
                    if e + 2 < NE:
                        xt_bufs[e + 2] = BX.tile([128, NDC, ET], bf16,
                                                 tag="xt", name=f"xt{e + 2}")
                        for g in range(2):
                            nc.sync.dma_start(
                                out=xt_bufs[e + 2][:, ts(g, 8), :],
                                in_=xtr[:, ts(g, 8), (e + 2) * ET:(e + 3) * ET])
                    xt_t = xt_bufs.pop(e)
                    # K projection
                    kp = PP.tile([128, ET], f32, tag="pp", name="kp")
                    for c in range(NDC):
                        nc.tensor.matmul(kp, wk_sb[:, c, :], xt_t[:, c, :],
                                         start=(c == 0), stop=(c == NDC - 1))

                    def vproj():
                        for i in range(2):
                            vp = PP.tile([128, HD], f32, tag="pp",
                                         name=f"vp{i}")
                            for c in range(NDC):
                                nc.tensor.matmul(vp, xt_t[:, c, ts(i, 128)],
                                                 wv_sb[:, c, :],
                                                 start=(c == 0),
                                                 stop=(c == NDC - 1))
                            nc.scalar.activation(Vn[:, 2 * e + i, :], vp,
                                                 AF.Copy)

                    # V projections right after K, except on the last eighth
                    # where they go last: PE-only work that covers the final
                    # rope chain's DVE drain at the phase boundary.
                    if e < NE - 1:
                        vproj()
                    normrope(kp, wkc, wke, sl, KT[:, sl])
                    # Q heads
                    for h in range(GQ):
                        qp = PP.tile([128, ET], f32, tag="pp", name=f"qp{h}")
                        for c in range(NDC):
                            nc.tensor.matmul(qp, wq_sb[:, c, ts(h, 128)],
                                             xt_t[:, c, :],
                                             start=(c == 0), stop=(c == NDC - 1))
                        normrope(qp, wqc, wqe, sl, QT[:, h, sl])
                    if e == NE - 1:
                        vproj()

            # ---------------- phase 2: attention + out projection ---------
            with (
                tc.tile_pool(name="PS2", bufs=3, space="PSUM") as PS2,
                tc.tile_pool(name="PLP", bufs=1, space="PSUM") as PLP,
                tc.tile_pool(name="POP", bufs=1, space="PSUM") as POP,
                tc.tile_pool(name="PS3", bufs=3, space="PSUM") as PS3,
            ):
                def outproj_chunk(Jm, c, spread=False):
                    """Output projection: column chunk c (of 4) for the four
                    128-row q tiles of block Jm; streams each [128,512] chunk
                    straight out. spread=True rotates oup allocations through
                    the idle lp/op banks too (used for the final block, which
                    has no attention work to hide the bank latency)."""
                    for qt in range(4):
                        qtg = 4 * Jm + qt
                        if spread and qt == 1:
                            oup = PLP.tile([128, 512], f32, tag="lp")
                        elif spread and qt == 2:
                            oup = POP.tile([128, 512], f32, tag="op")
                        else:
                            oup = PS3.tile([128, 512], f32, tag="oup")
                        for hc in range(GQ):
                            nc.tensor.matmul(oup, OTJ[Jm % 2][:, hc, ts(qt, 128)],
                                             wo_sb[:, hc, ts(c, 512)],
                                             start=(hc == 0), stop=(hc == GQ - 1))
                        oc = CO.tile([128, 512], f32, tag="oc")
                        nc.scalar.activation(oc, oup, AF.Copy)
                        nc.sync.dma_start(
                            out=outd[qtg * 128:(qtg + 1) * 128,
                                     c * 512:(c + 1) * 512],
                            in_=oc)

                for J in range(NJ):
                    nkb = 4 * J + 4
                    for h in range(GQ):
                        lp = PLP.tile([128, JW], f32, tag="lp")
                        op = POP.tile([128, JW], f32, tag="op")
                        for kb in range(nkb):
                            r = kb - 4 * J  # >= 0 on the diagonal blocks
                            c0 = 128 * r if r > 0 else 0
                            sp = PS2.tile([128, JW], f32, tag="s")
                            nc.tensor.matmul(sp[:, c0:JW], KT[:, ts(kb, 128)],
                                             QT[:, h, J * JW + c0:(J + 1) * JW],
                                             start=True, stop=True,
                                             skip_group_check=True)
                            P = CP.tile([128, JW], bf16, tag="p")
                            nc.scalar.activation(P[:, c0:JW], sp[:, c0:JW],
                                                 AF.Exp, scale=SCALE)
                            if r >= 0:
                                nc.vector.tensor_mul(
                                    P[:, 128 * r:128 * r + 128],
                                    P[:, 128 * r:128 * r + 128], tri_sb)
                            nc.tensor.matmul(lp[:, c0:JW], ones_sb, P[:, c0:JW],
                                             start=(kb == 0), stop=(kb == nkb - 1),
                                             skip_group_check=True)
                            nc.tensor.matmul(op[:, c0:JW], Vn[:, kb, :],
                                             P[:, c0:JW],
                                             start=(kb == 0), stop=(kb == nkb - 1),
                                             skip_group_check=True)
                        rc2 = CT.tile([128, JW], f32, tag="rc2")
                        nc.vector.reciprocal(rc2, lp)
                        nc.vector.tensor_mul(OTJ[J % 2][:, h, :], op, rc2)
                        if J > 0:
                            outproj_chunk(J - 1, h)
                for c in range(4):
                    outproj_chunk(NJ - 1, c)

    nc.finalize()
    return nc


def _host_consts():
    import ml_dtypes
    bf16 = ml_dtypes.bfloat16
    inv = 1.0 / (ROPE_BASE ** (np.arange(0, HD, 2, dtype=np.float64) / HD))
    freqs = np.outer(np.arange(T, dtype=np.float64), inv)
    emb = np.concatenate([freqs, freqs], axis=-1)          # [T, HD]
    cosT = np.ascontiguousarray(np.cos(emb).T.astype(bf16))  # [HD, T]
    sinT = np.ascontiguousarray(np.sin(emb).T.astype(bf16))
    # signed permutation for rotate-half: qr[m] = -qn[m+64] (m<64), qn[m-64]
    perm = np.zeros((128, 128), np.float32)
    for m in range(64):
        perm[m + 64, m] = -1.0
        perm[m, m + 64] = 1.0
    perm = perm.astype(bf16)
    k = np.arange(128)[:, None]
    q = np.arange(128)[None, :]
    tri = (k <= q).astype(bf16)
    ones = np.ones((128, 128), bf16)
    onesn = np.full((128, 128), 1.0 / HD, bf16)
    return cosT, sinT, perm, tri, ones, onesn


def kernel(x, Wq, Wk, Wv, Wo, q_norm_w, k_norm_w):
    import ml_dtypes
    from concourse.bass_utils import run_bass_kernel_spmd
    bf16 = ml_dtypes.bfloat16

    if "nc" not in _cached:
        _cached["nc"] = _build_program()
        _cached["consts"] = _host_consts()
    nc = _cached["nc"]
    cosT, sinT, perm, tri, ones, onesn = _cached["consts"]

    x = np.asarray(x, np.float32)
    Wq = np.asarray(Wq, np.float32)
    Wk = np.asarray(Wk, np.float32)
    Wv = np.asarray(Wv, np.float32)
    Wo = np.asarray(Wo, np.float32)
    qwf = np.asarray(q_norm_w, np.float64).reshape(HD, 1)
    kwf = np.asarray(k_norm_w, np.float64).reshape(HD, 1)
    qw = np.ascontiguousarray((1.0 / qwf ** 2).astype(np.float32))
    kw = np.ascontiguousarray((1.0 / kwf ** 2).astype(np.float32))
    qwe = np.ascontiguousarray((EPS / qwf ** 2).astype(np.float32))
    kwe = np.ascontiguousarray((EPS / kwf ** 2).astype(np.float32))

    xTb = [np.ascontiguousarray(x[b].T.astype(bf16)) for b in range(B)]
    in_maps = []
    for core in range(8):
        b, kv = divmod(core, NKV)
        wq_p = np.ascontiguousarray(
            Wq[:, kv * HQ:(kv + 1) * HQ].reshape(NDC, 128, HQ)
            .transpose(1, 0, 2).astype(bf16))
        wk_p = np.ascontiguousarray(
            Wk[:, kv * HD:(kv + 1) * HD].reshape(NDC, 128, HD)
            .transpose(1, 0, 2).astype(bf16))
        wv_p = np.ascontiguousarray(
            Wv[:, kv * HD:(kv + 1) * HD].reshape(NDC, 128, HD)
            .transpose(1, 0, 2).astype(bf16))
        wo_p = np.ascontiguousarray(
            Wo[kv * HQ:(kv + 1) * HQ, :].reshape(GQ, 128, D)
            .transpose(1, 0, 2).astype(bf16))
        in_maps.append({
            "xt": xTb[b],
            "wq": wq_p, "wk": wk_p, "wv": wv_p, "wo": wo_p,
            "cos": cosT, "sin": sinT,
            "perm": perm, "tri": tri, "ones": ones, "onesn": onesn,
            "wqc": qw, "wkc": kw, "wqe": qwe, "wke": kwe,
        })
    res = run_bass_kernel_spmd(nc, in_maps, list(range(8)))
    out = np.zeros((B, T, D), np.float64)
    for core in range(8):
        b = core // NKV
        out[b] += res.results[core]["out"].astype(np.float64)
    return out.astype(np.float32)
